# revision 39
# baseline (speedup 1.0000x reference)
"""Distributed multi-head attention for 8 TRN2 NeuronCores.

Problem: x[2,2048,1024] -> QKV proj (w_qkv[3072,1024]) -> 16-head SDPA ->
out proj (w_proj[1024,1024] + b_proj) -> [2,2048,1024].

Sharding: 2 heads per core (head-parallel over all 8 cores; both batches on
every core); output token rows split so core j owns tokens
[b*2048 + half*1024 + j*128, +128) for every (batch, half) -- i.e. 128
tokens per half-batch, 512 rows total.

Per core schedule (single persistent TileContext, pools never close):
  QKV(b): 12 groups of 8 accumulating matmuls -> qT/kT [128, 2048] per
          batch; V-natural via PE transpose with a ones row appended
          ([V|1], 65 cols per head) so P@[V|1] yields the softmax
          denominator for free in row 64.
  ATTN(b): per (qchunk, key-tile): S^T = kT.T @ qT (two K=64 row-tiled
          matmuls), P = exp(S/8) on the scalar engine, O^T[65,512]
          accumulated in PSUM. At the last key tile the UNNORMALIZED
          [65,512] block (row 64 = denominator) is cast to bf16 and
          DMA'd into the per-half-batch AllToAll staging buffer.
  4 AllToAlls (one per (batch, half)), each triggered as soon as its
          1024 tokens are staged, so all but the last overlap compute.
  PhaseC(hb): after A2A hb lands: batched reciprocal of the 16
          denominators [16,128], gpsimd partition-broadcast + DVE
          multiply to normalize, then out = attnT.T @ w_proj.T + bias
          for this core's 128 tokens. PhaseC work and QKV(b=1) are
          interleaved as fillers inside the scalar-bound attention
          loops to keep the PE continuously busy (p-state).
Host gathers: per core 4 sections of 128 token rows -> [2,2048,1024].
"""
import sys, os, types
import numpy as np

if "/opt/trn_rl_repo" not in sys.path and os.path.isdir("/opt/trn_rl_repo"):
    sys.path.append("/opt/trn_rl_repo")

import concourse.bass as bass
import concourse.mybir as mybir
import concourse.tile as tile
from concourse import bacc
from concourse.bass_utils import run_bass_kernel_spmd

F32 = mybir.dt.float32
BF16 = mybir.dt.bfloat16
EXP = mybir.ActivationFunctionType.Exp

NCORES = 8
B, N, C, H, D = 2, 2048, 1024, 16, 64
NT = B * N          # 4096 flat tokens
KT = C // 128       # 8 contraction tiles of 128
QC = 512            # query-chunk width
NMT = N // 128      # 16 key tiles per batch
SCALE = 1.0 / 8.0   # 1/sqrt(D)
XCH = 512           # x load chunk width
NXC = N // XCH      # 4 chunks per batch
TOK = 128           # tokens owned per core per half-batch
NHB = 4             # half-batches (a2a units)

TRACE = False       # test harness sets True to capture exec_time_ns
LAST_EXEC_NS = None

_NC = None


def _install_ntff_hook():
    if "antenv.axon_hooks" in sys.modules:
        return
    try:
        import antenv
        from trn_agent_boot.trn_boot import _ntff_profile_via_ctypes
        mod = types.ModuleType("antenv.axon_hooks")
        _hook = [None]
        mod.set_axon_ntff_profile_hook = lambda h: _hook.__setitem__(0, h)
        mod.get_axon_ntff_profile_hook = lambda: _hook[0]
        sys.modules["antenv.axon_hooks"] = mod
        antenv.axon_hooks = mod
        mod.set_axon_ntff_profile_hook(
            _ntff_profile_via_ctypes("/opt/axon/libaxon_pjrt.so"))
    except Exception:
        pass


def _build():
    nc = bacc.Bacc("TRN2", target_bir_lowering=False, debug=False,
                   num_devices=NCORES)
    xT_ext = nc.dram_tensor("xT", [C, NT], BF16, kind="ExternalInput").ap()
    wT_ext = nc.dram_tensor("wT", [C, 384], BF16, kind="ExternalInput").ap()
    wpT_ext = nc.dram_tensor("wpT", [C, C], BF16, kind="ExternalInput").ap()
    bias_ext = nc.dram_tensor("bias", [1, C], F32, kind="ExternalInput").ap()
    idn_ext = nc.dram_tensor("idn", [128, 128], BF16, kind="ExternalInput").ap()
    sel_ext = nc.dram_tensor("sel", [16, KT * 128], BF16,
                             kind="ExternalInput").ap()
    out_ext = nc.dram_tensor("out", [NHB * TOK, C], BF16,
                             kind="ExternalOutput").ap()
    a2a_in = [nc.dram_tensor(f"a2a_in{i}", [NCORES * 130, TOK], BF16)
              for i in range(NHB)]
    a2a_out = [nc.dram_tensor(f"a2a_out{i}", [NCORES * 130, TOK], BF16)
               for i in range(NHB)]
    a2a_w = [nc.dram_tensor(f"a2a_w{i}", [8, 8], BF16) for i in range(2)]

    xT_v = xT_ext.rearrange("(kt p) n -> p kt n", p=128)
    wT_v = wT_ext.rearrange("(kt p) f -> p kt f", p=128)
    wpT_v = wpT_ext.rearrange("(kt p) f -> p kt f", p=128)

    with tile.TileContext(nc) as tc:
        with (
            tc.tile_pool(name="const", bufs=1) as cpool,
            tc.tile_pool(name="resid", bufs=1) as rpool,
            tc.tile_pool(name="xchunk", bufs=1) as xpool,
            tc.tile_pool(name="vtmp", bufs=2) as vpool,
            tc.tile_pool(name="pexp", bufs=4) as ppool,
            tc.tile_pool(name="ostg", bufs=4) as stpool,
            tc.tile_pool(name="cden", bufs=2) as dpool,
            tc.tile_pool(name="clhs", bufs=2) as lpool,

            tc.tile_pool(name="cout", bufs=4) as outpool,
            tc.tile_pool(name="spsum", bufs=2, space="PSUM") as spool,
            tc.tile_pool(name="opsum", bufs=1, space="PSUM") as opool,
            tc.tile_pool(name="mpsum", bufs=2, space="PSUM") as mpool,
        ):
            # warm-up collective: the first CC op pays ~11us of stream
            # setup; absorb it during QKV with a tiny dummy AllToAll
            nc.gpsimd.collective_compute(
                "AllToAll", mybir.AluOpType.bypass,
                replica_groups=[list(range(NCORES))],
                ins=[a2a_w[0].ap()], outs=[a2a_w[1].ap()])

            # ---- constants ----
            wT_sb = cpool.tile([128, KT, 384], BF16)
            for kt in range(KT):
                nc.sync.dma_start(wT_sb[:, kt, :], wT_v[:, kt, :])
            idn = cpool.tile([128, 128], BF16)
            nc.sync.dma_start(idn[:], idn_ext[:])
            bias_sb = cpool.tile([1, C], F32)
            nc.sync.dma_start(bias_sb[:], bias_ext[:])
            bias_bc = cpool.tile([128, C], F32)
            nc.gpsimd.partition_broadcast(bias_bc[:], bias_sb[:])
            sel_sb = cpool.tile([16, KT, 128], BF16)
            nc.sync.dma_start(sel_sb[:], sel_ext[:])

            # per-(batch, 512-token chunk) tiles so cross-batch reads
            # never pick up false whole-tile dependencies
            qT_sb = {(b, c): rpool.tile([128, XCH], BF16, name=f"qT{b}{c}")
                     for b in range(B) for c in range(NXC)}
            kT_sb = {(b, c): rpool.tile([128, XCH], BF16, name=f"kT{b}{c}")
                     for b in range(B) for c in range(NXC)}
            v_sb = {(b, c): rpool.tile([128, 4, 130], BF16,
                                       name=f"v{b}{c}")
                    for b in range(B) for c in range(NXC)}
            for b in range(B):
                for c in range(NXC):
                    nc.gpsimd.memset(v_sb[(b, c)][:, :, 64], 1.0)
                    nc.gpsimd.memset(v_sb[(b, c)][:, :, 129], 1.0)
            wp_sb = rpool.tile([128, KT, C], BF16)

            # ---- x loads: only batch-0 chunk 0 upfront so the first
            # QKV group isn't starved by bulk DMA; the rest is issued
            # in small doses from the compute schedule below ----
            x_tiles = {}
            for b in range(B):
                for nch in range(NXC):
                    x_tiles[(b, nch)] = xpool.tile(
                        [128, KT, XCH], BF16, tag=f"x{b}{nch}",
                        name=f"x_{b}_{nch}")

            def x_load(b, nch, hi=False):
                def f():
                    import contextlib
                    ctx = tc.high_priority() if hi else contextlib.nullcontext()
                    with ctx:
                        for kt in range(KT):
                            nc.sync.dma_start(
                                x_tiles[(b, nch)][:, kt, :],
                                xT_v[:, kt,
                                     b * N + nch * XCH:
                                     b * N + (nch + 1) * XCH])
                return f

            x_load(0, 0, hi=True)()

            def qkv_subs(b, nch, ft):
                """One QKV matmul group split into small filler closures:
                3x(2 or 3 accumulating matmuls) + evacuation (v-feature
                groups also emit the PE transposes building V-natural)."""
                ncol = nch * XCH
                xs = x_tiles[(b, nch)]
                st = {}

                def mms(k0, k1):
                    def f():
                        if k0 == 0:
                            st["ps"] = mpool.tile(
                                [128, QC], F32, tag="mm",
                                name=f"qkv_{b}_{ncol}_{ft}")
                        for kt in range(k0, k1):
                            nc.tensor.matmul(
                                st["ps"][:],
                                wT_sb[:, kt, ft * 128:(ft + 1) * 128],
                                xs[:, kt, :],
                                start=(kt == 0), stop=(kt == KT - 1))
                    return f

                def evac():
                    ps = st["ps"]
                    if ft == 0:
                        nc.vector.tensor_copy(
                            out=qT_sb[(b, nch)][:], in_=ps[:])
                    elif ft == 1:
                        nc.vector.tensor_copy(
                            out=kT_sb[(b, nch)][:], in_=ps[:])
                    else:
                        st["vt"] = vpool.tile([128, QC], BF16, tag="vt",
                                              name=f"vt_{b}_{ncol}")
                        nc.vector.tensor_copy(out=st["vt"][:], in_=ps[:])

                def trans(t0, t1):
                    def f():
                        for t in range(t0, t1):
                            trp = mpool.tile([128, 128], BF16, tag="mm",
                                             name=f"tr_{b}_{nch}_{t}")
                            nc.tensor.transpose(
                                trp[:], st["vt"][:, t * 128:(t + 1) * 128],
                                idn[:])
                            nc.vector.tensor_copy(
                                out=v_sb[(b, nch)][:, t, 0:64],
                                in_=trp[:, 0:64])
                            nc.vector.tensor_copy(
                                out=v_sb[(b, nch)][:, t, 65:129],
                                in_=trp[:, 64:128])
                    return f

                subs = [mms(0, 3), mms(3, 6), mms(6, 8)]
                if ft < 2:
                    subs.append(evac)
                else:
                    subs.append(lambda: (evac(), trans(0, 2)()))
                    subs.append(trans(2, 4))
                return subs

            def phase_c(hb):
                """Output projection for this core's 128 tokens of
                half-batch hb, split into filler closures. Returns the
                closure list; caller schedules them after A2A hb lands.
                DMA issues go on the gpsimd queue -- by the time a
                closure runs, its A2A must have landed or gpsimd stalls
                (delaying later collective triggers), so the caller
                leaves generous margin after the trigger."""
                ao = a2a_out[hb].ap()
                den_v = ao.rearrange("(j r) t -> j r t", r=130)

                den = dpool.tile([16, TOK], BF16, tag="den",
                                 name=f"den_{hb}")
                denf = dpool.tile([16, TOK], F32, tag="denf",
                                  name=f"denf_{hb}")
                rcp = dpool.tile([16, TOK], F32, tag="rcp",
                                 name=f"rcp_{hb}")
                rcpb = dpool.tile([16, TOK], BF16, tag="rcpb",
                                  name=f"rcpb_{hb}")
                lhs = lpool.tile([128, KT, TOK], BF16, tag="lhs",
                                 name=f"lhs_{hb}")
                lhs_n = lpool.tile([128, KT, TOK], BF16, tag="lhsn",
                                   name=f"lhsn_{hb}")
                rb_all = lpool.tile([128, KT, TOK], BF16, tag="rb",
                                    name=f"rb_{hb}")

                def c_dma():
                    # denominators: rows j*130 + h*65 + 64; head-dim
                    # rows gathered as two strided DMAs (one per local
                    # head) instead of 16 row-block DMAs
                    nc.gpsimd.dma_start(den[0:8, :], den_v[:, 64, :])
                    nc.gpsimd.dma_start(den[8:16, :], den_v[:, 129, :])
                    rjt = den_v.transpose((1, 0, 2))  # [130, j, t]
                    nc.gpsimd.dma_start(lhs[0:64, :, :], rjt[0:64, :, :])
                    nc.gpsimd.dma_start(lhs[64:128, :, :],
                                        rjt[65:129, :, :])

                def c_recip():
                    nc.vector.tensor_copy(out=denf[:], in_=den[:])
                    nc.vector.reciprocal(rcp[:], denf[:])
                    nc.vector.tensor_copy(out=rcpb[:], in_=rcp[:])

                pp = {}

                def c_chunk(k0, k1):
                    def f():
                        # broadcast rcp rows (kt, 8+kt) to [128, TOK]
                        # via a tiny selector matmul (engine APs can't
                        # start at odd partitions), normalize, then run
                        # the projection chain
                        for kt in range(k0, k1):
                            rb = spool.tile([128, TOK], F32, tag="s",
                                            name=f"rb_{hb}_{kt}")
                            nc.tensor.matmul(
                                rb[:], sel_sb[:, kt, :], rcpb[:],
                                start=True, stop=True)
                            nc.vector.tensor_tensor(
                                lhs_n[:, kt, :], lhs[:, kt, :], rb[:],
                                mybir.AluOpType.mult)
                        for half in range(2):
                            if k0 == 0:
                                pp[half] = mpool.tile(
                                    [128, QC], F32, tag="mm",
                                    name=f"pp_{hb}_{half}")
                            for kt in range(k0, k1):
                                nc.tensor.matmul(
                                    pp[half][:],
                                    lhs_n[:, kt, :],
                                    wp_sb[:, kt, half * QC:(half + 1) * QC],
                                    start=(kt == 0), stop=(kt == KT - 1))
                    return f

                def c_out():
                    for half in range(2):
                        ot = outpool.tile([TOK, QC], BF16, tag="ot",
                                          name=f"ot_{hb}_{half}")
                        nc.vector.tensor_tensor(
                            ot[:], pp[half][:],
                            bias_bc[0:TOK, half * QC:(half + 1) * QC],
                            mybir.AluOpType.add)
                        nc.gpsimd.dma_start(
                            out_ext[hb * TOK:(hb + 1) * TOK,
                                    half * QC:(half + 1) * QC],
                            ot[:])

                return [c_dma, c_recip, c_chunk(0, 2), c_chunk(2, 4),
                        c_chunk(4, 6), c_chunk(6, 8), c_out]

            def attn_phase(b, fillers):
                """Attention for batch b. fillers: ordered list of
                (earliest_step, closure); at most one closure runs per
                step once step >= earliest (keeps filler bursts small so
                the scalar exp stream never starves). Steps 0..63."""
                pend = []  # software-pipelined PV emission
                fq = list(fillers)

                def flush_pv():
                    for f in pend:
                        f()
                    pend.clear()

                o_cur = {}
                for q in range(NXC):
                    qcol = q * QC
                    for mt in range(NMT):
                        step = q * NMT + mt
                        s_t = spool.tile([128, 2, QC], F32, tag="s",
                                         name=f"s_{b}_{step}")
                        for h in range(2):
                            nc.tensor.matmul(
                                s_t[:, h, :],
                                kT_sb[(b, mt // 4)][
                                    h * 64:(h + 1) * 64,
                                    (mt % 4) * 128:(mt % 4 + 1) * 128],
                                qT_sb[(b, q)][h * 64:(h + 1) * 64, :],
                                start=True, stop=True)
                        flush_pv()
                        p_t = ppool.tile([128, 2, QC], BF16, tag="p",
                                         name=f"p_{b}_{step}")
                        nc.scalar.activation(p_t[:], s_t[:], EXP,
                                             scale=SCALE)

                        def pv(mt=mt, q=q, p_t=p_t):
                            for h in range(2):
                                if mt == 0:
                                    o_cur[h] = opool.tile(
                                        [65, QC], F32, tag=f"o{h}",
                                        name=f"o_{b}_{q}_{h}")
                                nc.tensor.matmul(
                                    o_cur[h][:],
                                    v_sb[(b, mt // 4)][
                                        :, mt % 4, h * 65:(h + 1) * 65],
                                    p_t[:, h, :],
                                    start=(mt == 0), stop=(mt == NMT - 1))
                                if mt == NMT - 1:
                                    o_ps = o_cur.pop(h)
                                    stg = stpool.tile(
                                        [65, QC], BF16, tag="st",
                                        name=f"st_{b}_{q}_{h}")
                                    nc.vector.tensor_copy(out=stg[:],
                                                          in_=o_ps[:])
                                    hb = b * 2 + q // 2
                                    for dd in range(4):
                                        j = (q % 2) * 4 + dd
                                        nc.sync.dma_start(
                                            a2a_in[hb][
                                                j * 130 + h * 65:
                                                j * 130 + (h + 1) * 65, :],
                                            stg[:, dd * TOK:(dd + 1) * TOK])
                        pend.append(pv)
                        if fq and fq[0][0] <= step:
                            fq.pop(0)[1]()
                    if q % 2 == 1:
                        hb = b * 2 + q // 2
                        flush_pv()
                        nc.gpsimd.collective_compute(
                            "AllToAll",
                            mybir.AluOpType.bypass,
                            replica_groups=[list(range(NCORES))],
                            ins=[a2a_in[hb].ap()],
                            outs=[a2a_out[hb].ap()],
                        )
                flush_pv()
                for _, f in fq:  # leftover fillers run at phase end
                    f()

            # ---- schedule ----
            # QKV b0, with the next x chunk's DMAs issued between groups
            for nch in range(NXC):
                for ft in range(3):
                    if ft == 0 and nch < NXC - 1:
                        x_load(0, nch + 1, hi=True)()
                    for sub in qkv_subs(0, nch, ft):
                        sub()

            # attn b0: exp-bound, only the b1 x-chunk / w_proj DMA
            # issues as fillers (keeps the scalar engine saturated)
            fill0 = []
            for nch in range(NXC):
                fill0.append((16 + 6 * nch, x_load(1, nch)))
            fill0.append((40, lambda: nc.sync.dma_start(wp_sb[:],
                                                        wpT_v[:])))
            attn_phase(0, fill0)

            # QKV b1 dense: the PE sustains its high p-state here,
            # whereas interleaving it into attention drops everything
            # to the mid clock
            for nch in range(NXC):
                for ft in range(3):
                    for sub in qkv_subs(1, nch, ft):
                        sub()

            # attn b1 fillers: phase C hb0 (A2A landed during b0) and
            # hb1 (triggered at b0 end, lands ~step 10). hb2's A2A
            # lands too late in this window to schedule safely; it goes
            # to the tail. Steps 22-63 stay clear so staging DMAs and
            # triggers are never delayed.
            fill1 = []
            for cl in phase_c(0):
                fill1.append((0, cl))
            for cl in phase_c(1):
                fill1.append((16, cl))
            attn_phase(1, fill1)

            # tail: hb2's projection (its A2A landed mid-b1) overlaps
            # the hb3 AllToAll flight; hb3's DMAs wait on gpsimd only
            for cl in phase_c(2):
                cl()
            for cl in phase_c(3):
                cl()
    nc.compile()
    return nc


def kernel(x, w_qkv, w_proj, b_proj):
    global _NC, LAST_EXEC_NS
    if _NC is None:
        _NC = _build()
    x = np.asarray(x, dtype=np.float32)
    w_qkv = np.asarray(w_qkv, dtype=np.float32)
    w_proj = np.asarray(w_proj, dtype=np.float32)
    b_proj = np.asarray(b_proj, dtype=np.float32)

    import ml_dtypes
    xT = np.ascontiguousarray(x.reshape(NT, C).T).astype(ml_dtypes.bfloat16)
    wpT = np.ascontiguousarray(w_proj.T).astype(ml_dtypes.bfloat16)
    bias = np.ascontiguousarray(b_proj.reshape(1, C))
    idn = np.eye(128, dtype=ml_dtypes.bfloat16)
    # rcp partition layout: rows 0..7 = h0 dens (head 2j), 8..15 = h1
    # dens (head 2j+1); channel block kt holds heads (2kt, 2kt+1)
    sel = np.zeros((16, KT * 128), dtype=np.float32)
    for kt in range(KT):
        sel[kt, kt * 128:kt * 128 + 64] = 1.0
        sel[8 + kt, kt * 128 + 64:kt * 128 + 128] = 1.0
    sel = sel.astype(ml_dtypes.bfloat16)
    in_maps = []
    for c in range(NCORES):
        blk = slice(128 * c, 128 * (c + 1))
        wT = np.ascontiguousarray(
            np.concatenate([w_qkv[0:C][blk], w_qkv[C:2 * C][blk],
                            w_qkv[2 * C:3 * C][blk]], axis=0).T).astype(
                ml_dtypes.bfloat16)
        in_maps.append({"xT": xT, "wT": wT, "wpT": wpT, "bias": bias,
                        "idn": idn, "sel": sel})

    if TRACE:
        _install_ntff_hook()
    res = run_bass_kernel_spmd(_NC, in_maps, core_ids=list(range(NCORES)),
                               trace=TRACE)
    LAST_EXEC_NS = res.exec_time_ns
    out = np.empty((B, N, C), dtype=np.float32)
    for j in range(NCORES):
        o = np.asarray(res.results[j]["out"]).astype(np.float32)
        for hb in range(NHB):
            b, half = hb // 2, hb % 2
            t0 = half * 1024 + j * TOK
            out[b, t0:t0 + TOK, :] = o[hb * TOK:(hb + 1) * TOK, :]
    return np.ascontiguousarray(out)


# revision 40
# speedup vs baseline: 1.0250x; 1.0250x over previous
"""Distributed multi-head attention for 8 TRN2 NeuronCores.

Problem: x[2,2048,1024] -> QKV proj (w_qkv[3072,1024]) -> 16-head SDPA ->
out proj (w_proj[1024,1024] + b_proj) -> [2,2048,1024].

Sharding: 2 heads per core (head-parallel over all 8 cores; both batches on
every core); output token rows split so core j owns tokens
[b*2048 + half*1024 + j*128, +128) for every (batch, half) -- i.e. 128
tokens per half-batch, 512 rows total.

Per core schedule (single persistent TileContext, pools never close):
  QKV(b): 12 groups of 8 accumulating matmuls -> qT/kT [128, 2048] per
          batch; V-natural via PE transpose with a ones row appended
          ([V|1], 65 cols per head) so P@[V|1] yields the softmax
          denominator for free in row 64.
  ATTN(b): per (qchunk, key-tile): S^T = kT.T @ qT (two K=64 row-tiled
          matmuls), P = exp(S/8) on the scalar engine, O^T[65,512]
          accumulated in PSUM. At the last key tile the UNNORMALIZED
          [65,512] block (row 64 = denominator) is cast to bf16 and
          DMA'd into the per-half-batch AllToAll staging buffer.
  4 AllToAlls (one per (batch, half)), each triggered as soon as its
          1024 tokens are staged, so all but the last overlap compute.
  PhaseC(hb): after A2A hb lands: batched reciprocal of the 16
          denominators [16,128], gpsimd partition-broadcast + DVE
          multiply to normalize, then out = attnT.T @ w_proj.T + bias
          for this core's 128 tokens. PhaseC work and QKV(b=1) are
          interleaved as fillers inside the scalar-bound attention
          loops to keep the PE continuously busy (p-state).
Host gathers: per core 4 sections of 128 token rows -> [2,2048,1024].
"""
import sys, os, types
import numpy as np

if "/opt/trn_rl_repo" not in sys.path and os.path.isdir("/opt/trn_rl_repo"):
    sys.path.append("/opt/trn_rl_repo")

import concourse.bass as bass
import concourse.mybir as mybir
import concourse.tile as tile
from concourse import bacc
from concourse.bass_utils import run_bass_kernel_spmd

F32 = mybir.dt.float32
BF16 = mybir.dt.bfloat16
EXP = mybir.ActivationFunctionType.Exp

NCORES = 8
B, N, C, H, D = 2, 2048, 1024, 16, 64
NT = B * N          # 4096 flat tokens
KT = C // 128       # 8 contraction tiles of 128
QC = 512            # query-chunk width
NMT = N // 128      # 16 key tiles per batch
SCALE = 1.0 / 8.0   # 1/sqrt(D)
XCH = 512           # x load chunk width
NXC = N // XCH      # 4 chunks per batch
TOK = 128           # tokens owned per core per half-batch
NHB = 4             # half-batches (a2a units)

TRACE = False       # test harness sets True to capture exec_time_ns
LAST_EXEC_NS = None

_NC = None


def _install_ntff_hook():
    if "antenv.axon_hooks" in sys.modules:
        return
    try:
        import antenv
        from trn_agent_boot.trn_boot import _ntff_profile_via_ctypes
        mod = types.ModuleType("antenv.axon_hooks")
        _hook = [None]
        mod.set_axon_ntff_profile_hook = lambda h: _hook.__setitem__(0, h)
        mod.get_axon_ntff_profile_hook = lambda: _hook[0]
        sys.modules["antenv.axon_hooks"] = mod
        antenv.axon_hooks = mod
        mod.set_axon_ntff_profile_hook(
            _ntff_profile_via_ctypes("/opt/axon/libaxon_pjrt.so"))
    except Exception:
        pass


def _build():
    nc = bacc.Bacc("TRN2", target_bir_lowering=False, debug=False,
                   num_devices=NCORES)
    xT_ext = nc.dram_tensor("xT", [C, NT], BF16, kind="ExternalInput").ap()
    wT_ext = nc.dram_tensor("wT", [C, 384], BF16, kind="ExternalInput").ap()
    wpT_ext = nc.dram_tensor("wpT", [C, C], BF16, kind="ExternalInput").ap()
    bias_ext = nc.dram_tensor("bias", [1, C], F32, kind="ExternalInput").ap()
    idn_ext = nc.dram_tensor("idn", [128, 128], BF16, kind="ExternalInput").ap()
    sel_ext = nc.dram_tensor("sel", [16, KT * 128], BF16,
                             kind="ExternalInput").ap()
    out_ext = nc.dram_tensor("out", [NHB * TOK, C], BF16,
                             kind="ExternalOutput").ap()
    a2a_in = [nc.dram_tensor(f"a2a_in{i}", [NCORES * 130, TOK], BF16)
              for i in range(NHB)]
    a2a_out = [nc.dram_tensor(f"a2a_out{i}", [NCORES * 130, TOK], BF16)
               for i in range(NHB)]
    a2a_w = [nc.dram_tensor(f"a2a_w{i}", [8, 8], BF16) for i in range(2)]

    xT_v = xT_ext.rearrange("(kt p) n -> p kt n", p=128)
    wT_v = wT_ext.rearrange("(kt p) f -> p kt f", p=128)
    wpT_v = wpT_ext.rearrange("(kt p) f -> p kt f", p=128)

    with tile.TileContext(nc) as tc:
        with (
            tc.tile_pool(name="const", bufs=1) as cpool,
            tc.tile_pool(name="resid", bufs=1) as rpool,
            tc.tile_pool(name="xchunk", bufs=1) as xpool,
            tc.tile_pool(name="vtmp", bufs=2) as vpool,
            tc.tile_pool(name="pexp", bufs=4) as ppool,
            tc.tile_pool(name="ostg", bufs=4) as stpool,
            tc.tile_pool(name="cden", bufs=2) as dpool,
            tc.tile_pool(name="clhs", bufs=2) as lpool,

            tc.tile_pool(name="cout", bufs=4) as outpool,
            tc.tile_pool(name="spsum", bufs=2, space="PSUM") as spool,
            tc.tile_pool(name="opsum", bufs=1, space="PSUM") as opool,
            tc.tile_pool(name="mpsum", bufs=2, space="PSUM") as mpool,
        ):
            # warm-up collective: the first CC op pays ~11us of stream
            # setup; absorb it during QKV with a tiny dummy AllToAll
            nc.gpsimd.collective_compute(
                "AllToAll", mybir.AluOpType.bypass,
                replica_groups=[list(range(NCORES))],
                ins=[a2a_w[0].ap()], outs=[a2a_w[1].ap()])

            # ---- constants ----
            wT_sb = cpool.tile([128, KT, 384], BF16)
            for kt in range(KT):
                nc.sync.dma_start(wT_sb[:, kt, :], wT_v[:, kt, :])
            idn = cpool.tile([128, 128], BF16)
            nc.sync.dma_start(idn[:], idn_ext[:])
            bias_sb = cpool.tile([1, C], F32)
            nc.sync.dma_start(bias_sb[:], bias_ext[:])
            bias_bc = cpool.tile([128, C], F32)
            nc.gpsimd.partition_broadcast(bias_bc[:], bias_sb[:])
            sel_sb = cpool.tile([16, KT, 128], BF16)
            nc.sync.dma_start(sel_sb[:], sel_ext[:])

            # per-(batch, 512-token chunk) tiles so cross-batch reads
            # never pick up false whole-tile dependencies
            qT_sb = {(b, c): rpool.tile([128, XCH], BF16, name=f"qT{b}{c}")
                     for b in range(B) for c in range(NXC)}
            kT_sb = {(b, c): rpool.tile([128, XCH], BF16, name=f"kT{b}{c}")
                     for b in range(B) for c in range(NXC)}
            v_sb = {(b, c): rpool.tile([128, 4, 130], BF16,
                                       name=f"v{b}{c}")
                    for b in range(B) for c in range(NXC)}
            for b in range(B):
                for c in range(NXC):
                    nc.gpsimd.memset(v_sb[(b, c)][:, :, 64], 1.0)
                    nc.gpsimd.memset(v_sb[(b, c)][:, :, 129], 1.0)
            wp_sb = rpool.tile([128, KT, C], BF16)

            # ---- x loads: only batch-0 chunk 0 upfront so the first
            # QKV group isn't starved by bulk DMA; the rest is issued
            # in small doses from the compute schedule below ----
            x_tiles = {}
            for b in range(B):
                for nch in range(NXC):
                    x_tiles[(b, nch)] = xpool.tile(
                        [128, KT, XCH], BF16, tag=f"x{b}{nch}",
                        name=f"x_{b}_{nch}")

            def x_load(b, nch, hi=False):
                def f():
                    import contextlib
                    ctx = tc.high_priority() if hi else contextlib.nullcontext()
                    with ctx:
                        for kt in range(KT):
                            nc.sync.dma_start(
                                x_tiles[(b, nch)][:, kt, :],
                                xT_v[:, kt,
                                     b * N + nch * XCH:
                                     b * N + (nch + 1) * XCH])
                return f

            x_load(0, 0, hi=True)()

            def qkv_subs(b, nch, ft):
                """One QKV matmul group split into small filler closures:
                3x(2 or 3 accumulating matmuls) + evacuation (v-feature
                groups also emit the PE transposes building V-natural)."""
                ncol = nch * XCH
                xs = x_tiles[(b, nch)]
                st = {}

                def mms(k0, k1):
                    def f():
                        if k0 == 0:
                            st["ps"] = mpool.tile(
                                [128, QC], F32, tag="mm",
                                name=f"qkv_{b}_{ncol}_{ft}")
                        for kt in range(k0, k1):
                            nc.tensor.matmul(
                                st["ps"][:],
                                wT_sb[:, kt, ft * 128:(ft + 1) * 128],
                                xs[:, kt, :],
                                start=(kt == 0), stop=(kt == KT - 1))
                    return f

                def evac():
                    ps = st["ps"]
                    if ft == 0:
                        nc.vector.tensor_copy(
                            out=qT_sb[(b, nch)][:], in_=ps[:])
                    elif ft == 1:
                        nc.vector.tensor_copy(
                            out=kT_sb[(b, nch)][:], in_=ps[:])
                    else:
                        st["vt"] = vpool.tile([128, QC], BF16, tag="vt",
                                              name=f"vt_{b}_{ncol}")
                        nc.vector.tensor_copy(out=st["vt"][:], in_=ps[:])

                def trans(t0, t1):
                    def f():
                        for t in range(t0, t1):
                            trp = mpool.tile([128, 128], BF16, tag="mm",
                                             name=f"tr_{b}_{nch}_{t}")
                            nc.tensor.transpose(
                                trp[:], st["vt"][:, t * 128:(t + 1) * 128],
                                idn[:])
                            nc.vector.tensor_copy(
                                out=v_sb[(b, nch)][:, t, 0:64],
                                in_=trp[:, 0:64])
                            nc.vector.tensor_copy(
                                out=v_sb[(b, nch)][:, t, 65:129],
                                in_=trp[:, 64:128])
                    return f

                subs = [mms(0, 3), mms(3, 6), mms(6, 8)]
                if ft < 2:
                    subs.append(evac)
                else:
                    subs.append(lambda: (evac(), trans(0, 2)()))
                    subs.append(trans(2, 4))
                return subs

            def phase_c(hb):
                """Output projection for this core's 128 tokens of
                half-batch hb, split into filler closures. Returns the
                closure list; caller schedules them after A2A hb lands.
                DMA issues go on the gpsimd queue -- by the time a
                closure runs, its A2A must have landed or gpsimd stalls
                (delaying later collective triggers), so the caller
                leaves generous margin after the trigger."""
                ao = a2a_out[hb].ap()
                den_v = ao.rearrange("(j r) t -> j r t", r=130)

                den = dpool.tile([16, TOK], BF16, tag="den",
                                 name=f"den_{hb}")
                denf = dpool.tile([16, TOK], F32, tag="denf",
                                  name=f"denf_{hb}")
                rcp = dpool.tile([16, TOK], F32, tag="rcp",
                                 name=f"rcp_{hb}")
                rcpb = dpool.tile([16, TOK], BF16, tag="rcpb",
                                  name=f"rcpb_{hb}")
                lhs = lpool.tile([128, KT, TOK], BF16, tag="lhs",
                                 name=f"lhs_{hb}")
                lhs_n = lpool.tile([128, KT, TOK], BF16, tag="lhsn",
                                   name=f"lhsn_{hb}")
                rb_all = lpool.tile([128, KT, TOK], BF16, tag="rb",
                                    name=f"rb_{hb}")

                def c_dma():
                    # denominators: rows j*130 + h*65 + 64; head-dim
                    # rows gathered as two strided DMAs (one per local
                    # head) instead of 16 row-block DMAs
                    nc.gpsimd.dma_start(den[0:8, :], den_v[:, 64, :])
                    nc.gpsimd.dma_start(den[8:16, :], den_v[:, 129, :])
                    rjt = den_v.transpose((1, 0, 2))  # [130, j, t]
                    nc.gpsimd.dma_start(lhs[0:64, :, :], rjt[0:64, :, :])
                    nc.gpsimd.dma_start(lhs[64:128, :, :],
                                        rjt[65:129, :, :])

                def c_recip():
                    nc.vector.tensor_copy(out=denf[:], in_=den[:])
                    nc.vector.reciprocal(rcp[:], denf[:])
                    nc.vector.tensor_copy(out=rcpb[:], in_=rcp[:])

                pp = {}

                def c_chunk(k0, k1):
                    def f():
                        # broadcast rcp rows (kt, 8+kt) to [128, TOK]
                        # via a tiny selector matmul (engine APs can't
                        # start at odd partitions), normalize, then run
                        # the projection chain
                        for kt in range(k0, k1):
                            rb = spool.tile([128, TOK], F32, tag="s",
                                            name=f"rb_{hb}_{kt}")
                            nc.tensor.matmul(
                                rb[:], sel_sb[:, kt, :], rcpb[:],
                                start=True, stop=True)
                            nc.vector.tensor_tensor(
                                lhs_n[:, kt, :], lhs[:, kt, :], rb[:],
                                mybir.AluOpType.mult)
                        for half in range(2):
                            if k0 == 0:
                                pp[half] = mpool.tile(
                                    [128, QC], F32, tag="mm",
                                    name=f"pp_{hb}_{half}")
                            for kt in range(k0, k1):
                                nc.tensor.matmul(
                                    pp[half][:],
                                    lhs_n[:, kt, :],
                                    wp_sb[:, kt, half * QC:(half + 1) * QC],
                                    start=(kt == 0), stop=(kt == KT - 1))
                    return f

                def c_out():
                    for half in range(2):
                        ot = outpool.tile([TOK, QC], BF16, tag="ot",
                                          name=f"ot_{hb}_{half}")
                        nc.vector.tensor_tensor(
                            ot[:], pp[half][:],
                            bias_bc[0:TOK, half * QC:(half + 1) * QC],
                            mybir.AluOpType.add)
                        nc.gpsimd.dma_start(
                            out_ext[hb * TOK:(hb + 1) * TOK,
                                    half * QC:(half + 1) * QC],
                            ot[:])

                return [c_dma, c_recip, c_chunk(0, 2), c_chunk(2, 4),
                        c_chunk(4, 6), c_chunk(6, 8), c_out]

            def attn_phase(b, fillers):
                """Attention for batch b. fillers: ordered list of
                (earliest_step, closure); at most one closure runs per
                step once step >= earliest (keeps filler bursts small so
                the scalar exp stream never starves). Steps 0..63."""
                pend = []  # software-pipelined PV emission
                fq = list(fillers)

                def flush_pv():
                    for f in pend:
                        f()
                    pend.clear()

                o_cur = {}
                for q in range(NXC):
                    qcol = q * QC
                    for mt in range(NMT):
                        step = q * NMT + mt
                        s_t = spool.tile([128, 2, QC], F32, tag="s",
                                         name=f"s_{b}_{step}")
                        for h in range(2):
                            nc.tensor.matmul(
                                s_t[:, h, :],
                                kT_sb[(b, mt // 4)][
                                    h * 64:(h + 1) * 64,
                                    (mt % 4) * 128:(mt % 4 + 1) * 128],
                                qT_sb[(b, q)][h * 64:(h + 1) * 64, :],
                                start=True, stop=True)
                        flush_pv()
                        p_t = ppool.tile([128, 2, QC], BF16, tag="p",
                                         name=f"p_{b}_{step}")
                        nc.scalar.activation(p_t[:], s_t[:], EXP,
                                             scale=SCALE)

                        def pv(mt=mt, q=q, p_t=p_t):
                            for h in range(2):
                                if mt == 0:
                                    o_cur[h] = opool.tile(
                                        [65, QC], F32, tag=f"o{h}",
                                        name=f"o_{b}_{q}_{h}")
                                nc.tensor.matmul(
                                    o_cur[h][:],
                                    v_sb[(b, mt // 4)][
                                        :, mt % 4, h * 65:(h + 1) * 65],
                                    p_t[:, h, :],
                                    start=(mt == 0), stop=(mt == NMT - 1))
                                if mt == NMT - 1:
                                    o_ps = o_cur.pop(h)
                                    stg = stpool.tile(
                                        [65, QC], BF16, tag="st",
                                        name=f"st_{b}_{q}_{h}")
                                    nc.vector.tensor_copy(out=stg[:],
                                                          in_=o_ps[:])
                                    hb = b * 2 + q // 2
                                    for dd in range(4):
                                        j = (q % 2) * 4 + dd
                                        nc.sync.dma_start(
                                            a2a_in[hb][
                                                j * 130 + h * 65:
                                                j * 130 + (h + 1) * 65, :],
                                            stg[:, dd * TOK:(dd + 1) * TOK])
                        pend.append(pv)
                        if fq and fq[0][0] <= step:
                            fq.pop(0)[1]()
                    if q % 2 == 1:
                        hb = b * 2 + q // 2
                        flush_pv()
                        nc.gpsimd.collective_compute(
                            "AllToAll",
                            mybir.AluOpType.bypass,
                            replica_groups=[list(range(NCORES))],
                            ins=[a2a_in[hb].ap()],
                            outs=[a2a_out[hb].ap()],
                        )
                flush_pv()
                for _, f in fq:  # leftover fillers run at phase end
                    f()

            # ---- schedule ----
            # QKV b0, with the next x chunk's DMAs issued between groups
            for nch in range(NXC):
                for ft in range(3):
                    if ft == 0 and nch < NXC - 1:
                        x_load(0, nch + 1, hi=True)()
                    for sub in qkv_subs(0, nch, ft):
                        sub()

            # attn b0: exp-bound, only the b1 x-chunk / w_proj DMA
            # issues as fillers (keeps the scalar engine saturated)
            fill0 = []
            fill0.append((0, x_load(1, 0)))
            for nch in range(1, NXC):
                fill0.append((3 * nch, x_load(1, nch)))
            fill0.append((12, lambda: nc.sync.dma_start(wp_sb[:],
                                                        wpT_v[:])))
            attn_phase(0, fill0)

            # QKV b1 dense: the PE sustains its high p-state here,
            # whereas interleaving it into attention drops everything
            # to the mid clock
            for nch in range(NXC):
                for ft in range(3):
                    for sub in qkv_subs(1, nch, ft):
                        sub()

            # attn b1 fillers: phase C hb0 (A2A landed during b0) and
            # hb1 (triggered at b0 end, lands ~step 10). hb2's A2A
            # lands too late in this window to schedule safely; it goes
            # to the tail. Steps 22-63 stay clear so staging DMAs and
            # triggers are never delayed.
            fill1 = []
            for cl in phase_c(0):
                fill1.append((0, cl))
            for cl in phase_c(1):
                fill1.append((16, cl))
            attn_phase(1, fill1)

            # tail: hb2's projection (its A2A landed mid-b1) overlaps
            # the hb3 AllToAll flight; hb3's DMAs wait on gpsimd only
            for cl in phase_c(2):
                cl()
            for cl in phase_c(3):
                cl()
    nc.compile()
    return nc


def kernel(x, w_qkv, w_proj, b_proj):
    global _NC, LAST_EXEC_NS
    if _NC is None:
        _NC = _build()
    x = np.asarray(x, dtype=np.float32)
    w_qkv = np.asarray(w_qkv, dtype=np.float32)
    w_proj = np.asarray(w_proj, dtype=np.float32)
    b_proj = np.asarray(b_proj, dtype=np.float32)

    import ml_dtypes
    xT = np.ascontiguousarray(x.reshape(NT, C).T).astype(ml_dtypes.bfloat16)
    wpT = np.ascontiguousarray(w_proj.T).astype(ml_dtypes.bfloat16)
    bias = np.ascontiguousarray(b_proj.reshape(1, C))
    idn = np.eye(128, dtype=ml_dtypes.bfloat16)
    # rcp partition layout: rows 0..7 = h0 dens (head 2j), 8..15 = h1
    # dens (head 2j+1); channel block kt holds heads (2kt, 2kt+1)
    sel = np.zeros((16, KT * 128), dtype=np.float32)
    for kt in range(KT):
        sel[kt, kt * 128:kt * 128 + 64] = 1.0
        sel[8 + kt, kt * 128 + 64:kt * 128 + 128] = 1.0
    sel = sel.astype(ml_dtypes.bfloat16)
    in_maps = []
    for c in range(NCORES):
        blk = slice(128 * c, 128 * (c + 1))
        wT = np.ascontiguousarray(
            np.concatenate([w_qkv[0:C][blk], w_qkv[C:2 * C][blk],
                            w_qkv[2 * C:3 * C][blk]], axis=0).T).astype(
                ml_dtypes.bfloat16)
        in_maps.append({"xT": xT, "wT": wT, "wpT": wpT, "bias": bias,
                        "idn": idn, "sel": sel})

    if TRACE:
        _install_ntff_hook()
    res = run_bass_kernel_spmd(_NC, in_maps, core_ids=list(range(NCORES)),
                               trace=TRACE)
    LAST_EXEC_NS = res.exec_time_ns
    out = np.empty((B, N, C), dtype=np.float32)
    for j in range(NCORES):
        o = np.asarray(res.results[j]["out"]).astype(np.float32)
        for hb in range(NHB):
            b, half = hb // 2, hb % 2
            t0 = half * 1024 + j * TOK
            out[b, t0:t0 + TOK, :] = o[hb * TOK:(hb + 1) * TOK, :]
    return np.ascontiguousarray(out)


# revision 42
# speedup vs baseline: 1.0334x; 1.0082x over previous
"""Distributed multi-head attention for 8 TRN2 NeuronCores.

Problem: x[2,2048,1024] -> QKV proj (w_qkv[3072,1024]) -> 16-head SDPA ->
out proj (w_proj[1024,1024] + b_proj) -> [2,2048,1024].

Sharding: 2 heads per core (head-parallel over all 8 cores; both batches on
every core); output token rows split so core j owns tokens
[b*2048 + half*1024 + j*128, +128) for every (batch, half) -- i.e. 128
tokens per half-batch, 512 rows total.

Per core schedule (single persistent TileContext, pools never close):
  QKV(b): 12 groups of 8 accumulating matmuls -> qT/kT [128, 2048] per
          batch; V-natural via PE transpose with a ones row appended
          ([V|1], 65 cols per head) so P@[V|1] yields the softmax
          denominator for free in row 64.
  ATTN(b): per (qchunk, key-tile): S^T = kT.T @ qT (two K=64 row-tiled
          matmuls), P = exp(S/8) on the scalar engine, O^T[65,512]
          accumulated in PSUM. At the last key tile the UNNORMALIZED
          [65,512] block (row 64 = denominator) is cast to bf16 and
          DMA'd into the per-half-batch AllToAll staging buffer.
  4 AllToAlls (one per (batch, half)), each triggered as soon as its
          1024 tokens are staged, so all but the last overlap compute.
  PhaseC(hb): after A2A hb lands: batched reciprocal of the 16
          denominators [16,128], gpsimd partition-broadcast + DVE
          multiply to normalize, then out = attnT.T @ w_proj.T + bias
          for this core's 128 tokens. PhaseC work and QKV(b=1) are
          interleaved as fillers inside the scalar-bound attention
          loops to keep the PE continuously busy (p-state).
Host gathers: per core 4 sections of 128 token rows -> [2,2048,1024].
"""
import sys, os, types
import numpy as np

if "/opt/trn_rl_repo" not in sys.path and os.path.isdir("/opt/trn_rl_repo"):
    sys.path.append("/opt/trn_rl_repo")

import concourse.bass as bass
import concourse.mybir as mybir
import concourse.tile as tile
from concourse import bacc
from concourse.bass_utils import run_bass_kernel_spmd

F32 = mybir.dt.float32
BF16 = mybir.dt.bfloat16
EXP = mybir.ActivationFunctionType.Exp

NCORES = 8
B, N, C, H, D = 2, 2048, 1024, 16, 64
NT = B * N          # 4096 flat tokens
KT = C // 128       # 8 contraction tiles of 128
QC = 512            # query-chunk width
NMT = N // 128      # 16 key tiles per batch
SCALE = 1.0 / 8.0   # 1/sqrt(D)
XCH = 512           # x load chunk width
NXC = N // XCH      # 4 chunks per batch
TOK = 128           # tokens owned per core per half-batch
NHB = 4             # half-batches (a2a units)

TRACE = False       # test harness sets True to capture exec_time_ns
LAST_EXEC_NS = None

_NC = None


def _install_ntff_hook():
    if "antenv.axon_hooks" in sys.modules:
        return
    try:
        import antenv
        from trn_agent_boot.trn_boot import _ntff_profile_via_ctypes
        mod = types.ModuleType("antenv.axon_hooks")
        _hook = [None]
        mod.set_axon_ntff_profile_hook = lambda h: _hook.__setitem__(0, h)
        mod.get_axon_ntff_profile_hook = lambda: _hook[0]
        sys.modules["antenv.axon_hooks"] = mod
        antenv.axon_hooks = mod
        mod.set_axon_ntff_profile_hook(
            _ntff_profile_via_ctypes("/opt/axon/libaxon_pjrt.so"))
    except Exception:
        pass


def _build():
    nc = bacc.Bacc("TRN2", target_bir_lowering=False, debug=False,
                   num_devices=NCORES)
    xT_ext = nc.dram_tensor("xT", [C, NT], BF16, kind="ExternalInput").ap()
    wT_ext = nc.dram_tensor("wT", [C, 384], BF16, kind="ExternalInput").ap()
    wpT_ext = nc.dram_tensor("wpT", [C, C], BF16, kind="ExternalInput").ap()
    bias_ext = nc.dram_tensor("bias", [1, C], F32, kind="ExternalInput").ap()
    idn_ext = nc.dram_tensor("idn", [128, 128], BF16, kind="ExternalInput").ap()
    sel_ext = nc.dram_tensor("sel", [16, KT * 128], BF16,
                             kind="ExternalInput").ap()
    out_ext = nc.dram_tensor("out", [NHB * TOK, C], BF16,
                             kind="ExternalOutput").ap()
    a2a_in = [nc.dram_tensor(f"a2a_in{i}", [NCORES * 130, TOK], BF16)
              for i in range(NHB)]
    a2a_out = [nc.dram_tensor(f"a2a_out{i}", [NCORES * 130, TOK], BF16)
               for i in range(NHB)]
    a2a_w = [nc.dram_tensor(f"a2a_w{i}", [8, 8], BF16) for i in range(2)]

    xT_v = xT_ext.rearrange("(kt p) n -> p kt n", p=128)
    wT_v = wT_ext.rearrange("(kt p) f -> p kt f", p=128)
    wpT_v = wpT_ext.rearrange("(kt p) f -> p kt f", p=128)

    with tile.TileContext(nc) as tc:
        with (
            tc.tile_pool(name="const", bufs=1) as cpool,
            tc.tile_pool(name="resid", bufs=1) as rpool,
            tc.tile_pool(name="xchunk", bufs=1) as xpool,
            tc.tile_pool(name="vtmp", bufs=2) as vpool,
            tc.tile_pool(name="pexp", bufs=4) as ppool,
            tc.tile_pool(name="ostg", bufs=4) as stpool,
            tc.tile_pool(name="cden", bufs=2) as dpool,
            tc.tile_pool(name="clhs", bufs=2) as lpool,

            tc.tile_pool(name="cout", bufs=4) as outpool,
            tc.tile_pool(name="spsum", bufs=2, space="PSUM") as spool,
            tc.tile_pool(name="opsum", bufs=1, space="PSUM") as opool,
            tc.tile_pool(name="mpsum", bufs=2, space="PSUM") as mpool,
        ):
            # warm-up collective: the first CC op pays ~11us of stream
            # setup; absorb it during QKV with a tiny dummy AllToAll
            nc.gpsimd.collective_compute(
                "AllToAll", mybir.AluOpType.bypass,
                replica_groups=[list(range(NCORES))],
                ins=[a2a_w[0].ap()], outs=[a2a_w[1].ap()])

            # ---- constants ----
            wT_sb = cpool.tile([128, KT, 384], BF16)
            for kt in range(KT):
                nc.sync.dma_start(wT_sb[:, kt, :], wT_v[:, kt, :])
            idn = cpool.tile([128, 128], BF16)
            nc.sync.dma_start(idn[:], idn_ext[:])
            bias_sb = cpool.tile([1, C], F32)
            nc.sync.dma_start(bias_sb[:], bias_ext[:])
            bias_bc = cpool.tile([128, C], F32)
            nc.gpsimd.partition_broadcast(bias_bc[:], bias_sb[:])
            sel_sb = cpool.tile([16, KT, 128], BF16)
            nc.sync.dma_start(sel_sb[:], sel_ext[:])

            # per-(batch, 512-token chunk) tiles so cross-batch reads
            # never pick up false whole-tile dependencies
            qT_sb = {(b, c): rpool.tile([128, XCH], BF16, name=f"qT{b}{c}")
                     for b in range(B) for c in range(NXC)}
            kT_sb = {(b, c): rpool.tile([128, XCH], BF16, name=f"kT{b}{c}")
                     for b in range(B) for c in range(NXC)}
            v_sb = {(b, c): rpool.tile([128, 4, 130], BF16,
                                       name=f"v{b}{c}")
                    for b in range(B) for c in range(NXC)}
            for b in range(B):
                for c in range(NXC):
                    nc.gpsimd.memset(v_sb[(b, c)][:, :, 64], 1.0)
                    nc.gpsimd.memset(v_sb[(b, c)][:, :, 129], 1.0)
            wp_sb = rpool.tile([128, KT, C], BF16)

            # ---- x loads: only batch-0 chunk 0 upfront so the first
            # QKV group isn't starved by bulk DMA; the rest is issued
            # in small doses from the compute schedule below ----
            x_tiles = {}
            for b in range(B):
                for nch in range(NXC):
                    x_tiles[(b, nch)] = xpool.tile(
                        [128, KT, XCH], BF16, tag=f"x{b}{nch}",
                        name=f"x_{b}_{nch}")

            def x_load(b, nch):
                def f():
                    for kt in range(KT):
                        nc.sync.dma_start(
                            x_tiles[(b, nch)][:, kt, :],
                            xT_v[:, kt,
                                 b * N + nch * XCH:
                                 b * N + (nch + 1) * XCH])
                return f

            x_load(0, 0)()

            def qkv_subs(b, nch, ft):
                """One QKV matmul group split into small filler closures:
                3x(2 or 3 accumulating matmuls) + evacuation (v-feature
                groups also emit the PE transposes building V-natural)."""
                ncol = nch * XCH
                xs = x_tiles[(b, nch)]
                st = {}

                def mms(k0, k1):
                    def f():
                        if k0 == 0:
                            st["ps"] = mpool.tile(
                                [128, QC], F32, tag="mm",
                                name=f"qkv_{b}_{ncol}_{ft}")
                        for kt in range(k0, k1):
                            nc.tensor.matmul(
                                st["ps"][:],
                                wT_sb[:, kt, ft * 128:(ft + 1) * 128],
                                xs[:, kt, :],
                                start=(kt == 0), stop=(kt == KT - 1))
                    return f

                def evac():
                    ps = st["ps"]
                    if ft == 0:
                        nc.vector.tensor_copy(
                            out=qT_sb[(b, nch)][:], in_=ps[:])
                    elif ft == 1:
                        nc.vector.tensor_copy(
                            out=kT_sb[(b, nch)][:], in_=ps[:])
                    else:
                        st["vt"] = vpool.tile([128, QC], BF16, tag="vt",
                                              name=f"vt_{b}_{ncol}")
                        nc.vector.tensor_copy(out=st["vt"][:], in_=ps[:])

                def trans(t0, t1):
                    def f():
                        for t in range(t0, t1):
                            trp = mpool.tile([128, 128], BF16, tag="mm",
                                             name=f"tr_{b}_{nch}_{t}")
                            nc.tensor.transpose(
                                trp[:], st["vt"][:, t * 128:(t + 1) * 128],
                                idn[:])
                            nc.vector.tensor_copy(
                                out=v_sb[(b, nch)][:, t, 0:64],
                                in_=trp[:, 0:64])
                            nc.vector.tensor_copy(
                                out=v_sb[(b, nch)][:, t, 65:129],
                                in_=trp[:, 64:128])
                    return f

                subs = [mms(0, 3), mms(3, 6), mms(6, 8)]
                if ft < 2:
                    subs.append(evac)
                else:
                    subs.append(lambda: (evac(), trans(0, 2)()))
                    subs.append(trans(2, 4))
                return subs

            def phase_c(hb):
                """Output projection for this core's 128 tokens of
                half-batch hb, split into filler closures. Returns the
                closure list; caller schedules them after A2A hb lands.
                DMA issues go on the gpsimd queue -- by the time a
                closure runs, its A2A must have landed or gpsimd stalls
                (delaying later collective triggers), so the caller
                leaves generous margin after the trigger."""
                ao = a2a_out[hb].ap()
                den_v = ao.rearrange("(j r) t -> j r t", r=130)

                den = dpool.tile([16, TOK], BF16, tag="den",
                                 name=f"den_{hb}")
                denf = dpool.tile([16, TOK], F32, tag="denf",
                                  name=f"denf_{hb}")
                rcp = dpool.tile([16, TOK], F32, tag="rcp",
                                 name=f"rcp_{hb}")
                rcpb = dpool.tile([16, TOK], BF16, tag="rcpb",
                                  name=f"rcpb_{hb}")
                lhs = lpool.tile([128, KT, TOK], BF16, tag="lhs",
                                 name=f"lhs_{hb}")
                lhs_n = lpool.tile([128, KT, TOK], BF16, tag="lhsn",
                                   name=f"lhsn_{hb}")
                rb_all = lpool.tile([128, KT, TOK], BF16, tag="rb",
                                    name=f"rb_{hb}")

                def c_dma():
                    # denominators: rows j*130 + h*65 + 64; head-dim
                    # rows gathered as two strided DMAs (one per local
                    # head) instead of 16 row-block DMAs
                    nc.gpsimd.dma_start(den[0:8, :], den_v[:, 64, :])
                    nc.gpsimd.dma_start(den[8:16, :], den_v[:, 129, :])
                    rjt = den_v.transpose((1, 0, 2))  # [130, j, t]
                    nc.gpsimd.dma_start(lhs[0:64, :, :], rjt[0:64, :, :])
                    nc.gpsimd.dma_start(lhs[64:128, :, :],
                                        rjt[65:129, :, :])

                def c_recip():
                    nc.vector.tensor_copy(out=denf[:], in_=den[:])
                    nc.vector.reciprocal(rcp[:], denf[:])
                    nc.vector.tensor_copy(out=rcpb[:], in_=rcp[:])

                pp = {}

                def c_chunk(k0, k1):
                    def f():
                        # broadcast rcp rows (kt, 8+kt) to [128, TOK]
                        # via a tiny selector matmul (engine APs can't
                        # start at odd partitions), normalize, then run
                        # the projection chain
                        for kt in range(k0, k1):
                            rb = spool.tile([128, TOK], F32, tag="s",
                                            name=f"rb_{hb}_{kt}")
                            nc.tensor.matmul(
                                rb[:], sel_sb[:, kt, :], rcpb[:],
                                start=True, stop=True)
                            nc.vector.tensor_tensor(
                                lhs_n[:, kt, :], lhs[:, kt, :], rb[:],
                                mybir.AluOpType.mult)
                        for half in range(2):
                            if k0 == 0:
                                pp[half] = mpool.tile(
                                    [128, QC], F32, tag="mm",
                                    name=f"pp_{hb}_{half}")
                            for kt in range(k0, k1):
                                nc.tensor.matmul(
                                    pp[half][:],
                                    lhs_n[:, kt, :],
                                    wp_sb[:, kt, half * QC:(half + 1) * QC],
                                    start=(kt == 0), stop=(kt == KT - 1))
                    return f

                def c_out():
                    for half in range(2):
                        ot = outpool.tile([TOK, QC], BF16, tag="ot",
                                          name=f"ot_{hb}_{half}")
                        nc.vector.tensor_tensor(
                            ot[:], pp[half][:],
                            bias_bc[0:TOK, half * QC:(half + 1) * QC],
                            mybir.AluOpType.add)
                        nc.gpsimd.dma_start(
                            out_ext[hb * TOK:(hb + 1) * TOK,
                                    half * QC:(half + 1) * QC],
                            ot[:])

                return [c_dma, c_recip, c_chunk(0, 2), c_chunk(2, 4),
                        c_chunk(4, 6), c_chunk(6, 8), c_out]

            def attn_phase(b, fillers):
                """Attention for batch b. fillers: ordered list of
                (earliest_step, closure); at most one closure runs per
                step once step >= earliest (keeps filler bursts small so
                the scalar exp stream never starves). Steps 0..63."""
                pend = []  # software-pipelined PV emission
                fq = list(fillers)

                def flush_pv():
                    for f in pend:
                        f()
                    pend.clear()

                o_cur = {}
                for q in range(NXC):
                    qcol = q * QC
                    for mt in range(NMT):
                        step = q * NMT + mt
                        s_t = spool.tile([128, 2, QC], F32, tag="s",
                                         name=f"s_{b}_{step}")
                        for h in range(2):
                            nc.tensor.matmul(
                                s_t[:, h, :],
                                kT_sb[(b, mt // 4)][
                                    h * 64:(h + 1) * 64,
                                    (mt % 4) * 128:(mt % 4 + 1) * 128],
                                qT_sb[(b, q)][h * 64:(h + 1) * 64, :],
                                start=True, stop=True)
                        flush_pv()
                        p_t = ppool.tile([128, 2, QC], BF16, tag="p",
                                         name=f"p_{b}_{step}")
                        nc.scalar.activation(p_t[:], s_t[:], EXP,
                                             scale=SCALE)

                        def pv(mt=mt, q=q, p_t=p_t):
                            for h in range(2):
                                if mt == 0:
                                    o_cur[h] = opool.tile(
                                        [65, QC], F32, tag=f"o{h}",
                                        name=f"o_{b}_{q}_{h}")
                                nc.tensor.matmul(
                                    o_cur[h][:],
                                    v_sb[(b, mt // 4)][
                                        :, mt % 4, h * 65:(h + 1) * 65],
                                    p_t[:, h, :],
                                    start=(mt == 0), stop=(mt == NMT - 1))
                                if mt == NMT - 1:
                                    o_ps = o_cur.pop(h)
                                    stg = stpool.tile(
                                        [65, QC], BF16, tag="st",
                                        name=f"st_{b}_{q}_{h}")
                                    nc.vector.tensor_copy(out=stg[:],
                                                          in_=o_ps[:])
                                    hb = b * 2 + q // 2
                                    for dd in range(4):
                                        j = (q % 2) * 4 + dd
                                        nc.sync.dma_start(
                                            a2a_in[hb][
                                                j * 130 + h * 65:
                                                j * 130 + (h + 1) * 65, :],
                                            stg[:, dd * TOK:(dd + 1) * TOK])
                        pend.append(pv)
                        if fq and fq[0][0] <= step:
                            fq.pop(0)[1]()
                    if q % 2 == 1:
                        hb = b * 2 + q // 2
                        flush_pv()
                        nc.gpsimd.collective_compute(
                            "AllToAll",
                            mybir.AluOpType.bypass,
                            replica_groups=[list(range(NCORES))],
                            ins=[a2a_in[hb].ap()],
                            outs=[a2a_out[hb].ap()],
                        )
                flush_pv()
                for _, f in fq:  # leftover fillers run at phase end
                    f()

            # ---- schedule ----
            # QKV b0, with the next x chunk's DMAs issued between groups
            for nch in range(NXC):
                for ft in range(3):
                    if ft == 1 and nch < NXC - 1:
                        x_load(0, nch + 1)()
                    for sub in qkv_subs(0, nch, ft):
                        sub()

            # attn b0: exp-bound, only the b1 x-chunk / w_proj DMA
            # issues as fillers (keeps the scalar engine saturated)
            fill0 = []
            fill0.append((0, x_load(1, 0)))
            for nch in range(1, NXC):
                fill0.append((3 * nch, x_load(1, nch)))
            fill0.append((12, lambda: nc.sync.dma_start(wp_sb[:],
                                                        wpT_v[:])))
            attn_phase(0, fill0)

            # QKV b1 dense: the PE sustains its high p-state here,
            # whereas interleaving it into attention drops everything
            # to the mid clock
            for nch in range(NXC):
                for ft in range(3):
                    for sub in qkv_subs(1, nch, ft):
                        sub()

            # attn b1 fillers: phase C hb0 (A2A landed during b0) and
            # hb1 (triggered at b0 end, lands ~step 10). hb2's A2A
            # lands too late in this window to schedule safely; it goes
            # to the tail. Steps 22-63 stay clear so staging DMAs and
            # triggers are never delayed.
            fill1 = []
            for cl in phase_c(0):
                fill1.append((0, cl))
            for cl in phase_c(1):
                fill1.append((16, cl))
            attn_phase(1, fill1)

            # tail: hb2's projection (its A2A landed mid-b1) overlaps
            # the hb3 AllToAll flight; hb3's DMAs wait on gpsimd only
            for cl in phase_c(2):
                cl()
            for cl in phase_c(3):
                cl()
    nc.compile()
    return nc


def kernel(x, w_qkv, w_proj, b_proj):
    global _NC, LAST_EXEC_NS
    if _NC is None:
        _NC = _build()
    x = np.asarray(x, dtype=np.float32)
    w_qkv = np.asarray(w_qkv, dtype=np.float32)
    w_proj = np.asarray(w_proj, dtype=np.float32)
    b_proj = np.asarray(b_proj, dtype=np.float32)

    import ml_dtypes
    xT = np.ascontiguousarray(x.reshape(NT, C).T).astype(ml_dtypes.bfloat16)
    wpT = np.ascontiguousarray(w_proj.T).astype(ml_dtypes.bfloat16)
    bias = np.ascontiguousarray(b_proj.reshape(1, C))
    idn = np.eye(128, dtype=ml_dtypes.bfloat16)
    # rcp partition layout: rows 0..7 = h0 dens (head 2j), 8..15 = h1
    # dens (head 2j+1); channel block kt holds heads (2kt, 2kt+1)
    sel = np.zeros((16, KT * 128), dtype=np.float32)
    for kt in range(KT):
        sel[kt, kt * 128:kt * 128 + 64] = 1.0
        sel[8 + kt, kt * 128 + 64:kt * 128 + 128] = 1.0
    sel = sel.astype(ml_dtypes.bfloat16)
    in_maps = []
    for c in range(NCORES):
        blk = slice(128 * c, 128 * (c + 1))
        wT = np.ascontiguousarray(
            np.concatenate([w_qkv[0:C][blk], w_qkv[C:2 * C][blk],
                            w_qkv[2 * C:3 * C][blk]], axis=0).T).astype(
                ml_dtypes.bfloat16)
        in_maps.append({"xT": xT, "wT": wT, "wpT": wpT, "bias": bias,
                        "idn": idn, "sel": sel})

    if TRACE:
        _install_ntff_hook()
    res = run_bass_kernel_spmd(_NC, in_maps, core_ids=list(range(NCORES)),
                               trace=TRACE)
    LAST_EXEC_NS = res.exec_time_ns
    out = np.empty((B, N, C), dtype=np.float32)
    for j in range(NCORES):
        o = np.asarray(res.results[j]["out"]).astype(np.float32)
        for hb in range(NHB):
            b, half = hb // 2, hb % 2
            t0 = half * 1024 + j * TOK
            out[b, t0:t0 + TOK, :] = o[hb * TOK:(hb + 1) * TOK, :]
    return np.ascontiguousarray(out)
